# revision 1
# baseline (speedup 1.0000x reference)
"""AUC pairwise loss kernel for Trainium2, SPMD over 8 NeuronCores.

Reference computation (N = 16384):
    pred = softmax(y_pred)[:, 1]                       # (N,)
    a_i  = pred_i + GAMMA   for rows with y_true == 1  ("neg" axis)
    b_j  = pred_j           for rows with y_true == 0  ("pos" axis)
    S2   = sum_{a_i > b_j} (a_i - b_j)^2,  C = #{a_i > b_j}
    auc  = S2 / max(C, 1)
    bce  = -mean(yt*clip(log pred, -100) + (1-yt)*clip(log(1-pred), -100))
    loss = ALPHA*bce + (1.0-ALPHA)*auc   (bce alone if C == 0)

Sharding (host): compact the two classes (the reference itself is a
pos/neg selection before the outer product) and SORT each class by the
logit z1-z0 (monotone in pred, so a pure permutation of input rows).
"a" rows (yt==1) are dealt round-robin by sorted rank across the 8
cores (~n_neg/8 each, padded to a multiple of 128; pads sort last with
a = A_PAD and are masked out of the final reductions); every core gets
all "b" rows (yt==0, sorted, padded -> b = B_PAD which no real a
exceeds).  BCE is row-sharded N/8 per core.

Sorting makes ind[q, i] = (a_i > b_q) a monotone staircase over the
(b-block c) x (a-chunk h) tile grid.  The host classifies each tile
from f64 sigmoid bounds over global rank windows (safety margin 1e-4,
identical for all cores):  all-0 tiles are skipped, all-1 tiles are
reduced with a constant ones weight (no indicator computed), and only
the ~(#blocks + #chunks) mixed tiles on the staircase boundary get an
exact DVE is_gt indicator in bf16.

PE consumes indicator (or ones) tiles as the stationary operand (bf16
FWL) against the moving [128, 7] weight view [1|bh|bm|bl|b2h|b2m|b2l]
(b and b^2 each split into 3 bf16 words for f32-grade precision),
accumulating (k_i, s1_i, s2_i) per a-slot in psum, laid out exactly
like the a tiles.  Epilogue: 8 tiny PE matmuls dot the psum columns
with masked a-polynomials [a^2*m, a*m, m], yielding S2 = sum_i
a_i^2*k_i - 2*a_i*s1_i + s2_i and C = sum_i k_i directly; the host
combines the 8 small outputs (plus sharded BCE partials) into the loss.
"""

import numpy as np

from concourse import bacc, bass, mybir, tile
from concourse.bass_utils import run_bass_kernel_spmd

N = 16384
NCORES = 8
P = 128
FB = N // P                  # free-dim cols for full-N [128, 128] tiles
GAMMA = 0.15
ALPHA = 0.6
# pads sort above all real values (a in (GAMMA, 1+GAMMA), b in (0, 1)),
# keeping the per-chunk/per-block bounds monotone; pad "a" slots are
# masked out of the reductions by ma, pad "b" slots satisfy a_real < 2.
A_PAD = 2.5
B_PAD = 2.0
EPS = 1e-4                   # host-vs-device sigmoid classification margin

F32 = mybir.dt.float32
BF16 = mybir.dt.bfloat16
AF = mybir.ActivationFunctionType
OP = mybir.AluOpType
NW = 7  # weight cols: ones, bh, bm, bl, b2h, b2m, b2l


def build_nc(A, B, h0, h1, debug=False, repeat=1):
    """A: per-core padded 'a' rows; B: padded 'b' rows.
    h0/h1: per b-block staircase spans -- chunks [0,h0) are all-0
    (skipped), [h0,h1) mixed (exact indicator), [h1,FS) all-1 (ones
    weight).  repeat>1 re-runs the main loop for slope benchmarking.
    """
    FS = A // P      # a chunks of 128
    NBLK = B // P    # b blocks of 128
    assert len(h0) == NBLK and len(h1) == NBLK

    nc = bacc.Bacc("TRN2", target_bir_lowering=False, debug=debug)

    yp_a = nc.dram_tensor("yp_a", [A, 2], F32, kind="ExternalInput")
    ma = nc.dram_tensor("ma", [A], F32, kind="ExternalInput")      # 1=real
    ma_g = nc.dram_tensor("ma_g", [A], F32, kind="ExternalInput")  # gamma/pad
    yp_b = nc.dram_tensor("yp_b", [B, 2], F32, kind="ExternalInput")
    mb = nc.dram_tensor("mb", [B], F32, kind="ExternalInput")      # 1=pad
    yp_s = nc.dram_tensor("yp_s", [N // NCORES, 2], F32, kind="ExternalInput")
    yt_s = nc.dram_tensor("yt_s", [N // NCORES], F32, kind="ExternalInput")
    idn_d = nc.dram_tensor("idn", [P, P], F32, kind="ExternalInput")
    out = nc.dram_tensor("out", [P, 8 + 3 * FS], F32, kind="ExternalOutput")

    with tile.TileContext(nc) as tc:
        with (
            tc.tile_pool(name="const", bufs=1) as cpool,
            tc.tile_pool(name="work", bufs=2) as wpool,
            tc.tile_pool(name="ind", bufs=8) as ipool,
            tc.tile_pool(name="psum", bufs=1, space=bass.MemorySpace.PSUM) as ppool,
        ):
            ones2 = cpool.tile([2, P], F32)
            nc.vector.memset(ones2[:], 1.0)
            ones128 = cpool.tile([P, P], BF16)
            nc.vector.memset(ones128[:], 1.0)
            # identity for PE transpose (host-provided constant);
            # gpsimd queue = second parallel DMA stream (Pool is idle,
            # and dma_start is a native descriptor push, not ucode)
            idn = cpool.tile([P, P], F32)
            nc.gpsimd.dma_start(idn[:], idn_d[:])
            # ---------------- a side: neg pred + pad, broadcast ------------
            zab = wpool.tile([P, 2 * FS], F32)  # interleaved (z0, z1) cols
            nc.sync.dma_start(
                zab[:], yp_a[:].rearrange("(f p) c -> p f c", p=P)
            )
            zab_v = zab[:].rearrange("p (f c) -> p c f", c=2)
            mag = wpool.tile([P, FS], F32)
            nc.sync.dma_start(mag[:], ma_g[:].rearrange("(f p) -> p f", p=P))
            zs = wpool.tile([P, FS], F32)
            nc.vector.tensor_sub(zs[:], zab_v[:, 1, :], zab_v[:, 0, :])
            psig = wpool.tile([P, FS], F32)
            nc.scalar.activation(psig[:], zs[:], AF.Sigmoid)
            # am = sigmoid + maG (maG = GAMMA on real slots; pads have
            # z = 0 -> sigmoid 0.5 and maG = A_PAD - 0.5)
            am = cpool.tile([P, FS], F32)
            nc.vector.tensor_add(am[:], psig[:], mag[:])

            # a_bc[p, i] = sigmoid_i + maG_i: per-column PE transposes land
            # sigmoid [1, 128] segments in psum partition 0, a K=1 fp32
            # matmul broadcasts the row (exact, no gpsimd ucode), and the
            # psum->sbuf step adds the host-broadcast maG (GAMMA on real
            # slots; pads have z = 0 -> sigmoid 0.5, maG = A_PAD - 0.5).
            psum_t = ppool.tile([1, A], F32)
            for h in range(FS):
                nc.tensor.transpose(
                    psum_t[0:1, h * P:(h + 1) * P], am[:, h:h + 1], idn[:]
                )
            a_row = cpool.tile([1, A], F32)
            psum_bc = ppool.tile([P, A], F32)
            for j in range(0, A, 256):
                w = min(256, A - j)
                nc.vector.tensor_copy(a_row[0:1, j:j + w], psum_t[0:1, j:j + w])
                nc.tensor.matmul(
                    psum_bc[:, j:j + w], ones2[0:1, :], a_row[0:1, j:j + w],
                    start=True, stop=True,
                )
            a_bc = cpool.tile([P, A], F32)
            nc.scalar.copy(a_bc[:, :A // 2], psum_bc[:, :A // 2])
            nc.scalar.copy(a_bc[:, A // 2:], psum_bc[:, A // 2:])

            # ---------------- b side: pos pred + pad, weights --------------
            # layout: tile[p, f] = vec[f*128 + p]; one interleaved DMA
            zbb = wpool.tile([P, 2 * NBLK], F32)
            nc.gpsimd.dma_start(
                zbb[:], yp_b[:].rearrange("(f p) c -> p f c", p=P)
            )
            zbb_v = zbb[:].rearrange("p (f c) -> p c f", c=2)
            mbt = wpool.tile([P, NBLK], F32)
            nc.gpsimd.dma_start(mbt[:], mb[:].rearrange("(f p) -> p f", p=P))

            zb = wpool.tile([P, NBLK], F32)
            nc.vector.tensor_sub(zb[:], zbb_v[:, 1, :], zbb_v[:, 0, :])
            pbb = wpool.tile([P, NBLK], F32)
            nc.scalar.activation(pbb[:], zb[:], AF.Sigmoid)

            # bm = pbb + mbt * (B_PAD - pbb)
            tmb = wpool.tile([P, NBLK], F32)
            nc.vector.tensor_scalar(tmb[:], pbb[:], -1.0, B_PAD,
                                    op0=OP.mult, op1=OP.add)
            vmb = wpool.tile([P, NBLK], F32)
            nc.vector.tensor_mul(vmb[:], tmb[:], mbt[:])
            bm = cpool.tile([P, NBLK], F32)
            nc.vector.tensor_add(bm[:], pbb[:], vmb[:])
            bneg = cpool.tile([P, NBLK], F32)
            nc.scalar.mul(bneg[:], bm[:], -1.0)


            # bf16 weights [ones | -2bh | -2bm | -2bl | b2h | b2m | b2l];
            # the -2 scale (exact in bf16) pre-folds the cross term of
            # (a-b)^2 so the epilogue needs no extra multiply.  b^2 on DVE:
            # an ACT Square would force a 1.3us activation-table switch.
            b2 = cpool.tile([P, NBLK], F32)
            nc.vector.tensor_mul(b2[:], bm[:], bm[:])
            rhs_all = cpool.tile([P, NW * NBLK], BF16)
            nc.vector.memset(rhs_all[:, 0:NBLK], 1.0)
            for base, src, scl in ((1, bm, -2.0), (4, b2, 1.0)):
                resid = src
                for k in range(3):
                    dst = rhs_all[:, (base + k) * NBLK:(base + k + 1) * NBLK]
                    nc.vector.tensor_scalar(dst, resid[:], scl, None,
                                            op0=OP.mult)
                    if k < 2:
                        back = wpool.tile([P, NBLK], F32, name=f"back{base}{k}",
                                          tag="back")
                        nc.scalar.mul(back[:], dst, 1.0 / scl)
                        nresid = wpool.tile([P, NBLK], F32, name=f"res{base}{k}",
                                            tag="resid")
                        nc.vector.tensor_sub(nresid[:], resid[:], back[:])
                        resid = nresid

            # am = sigmoid + maG; masked a-polynomials [a^2*m | a*m | m]
            # for the epilogue dot products (pads contribute 0 via m)
            mat = wpool.tile([P, FS], F32)
            nc.sync.dma_start(mat[:], ma[:].rearrange("(f p) -> p f", p=P))
            am2 = wpool.tile([P, FS], F32)
            nc.vector.tensor_mul(am2[:], am[:], am[:])
            wpoly = cpool.tile([P, 3 * FS], F32)
            nc.vector.tensor_mul(wpoly[:, 0:FS], am2[:], mat[:])
            nc.vector.tensor_mul(wpoly[:, FS:2 * FS], am[:], mat[:])
            nc.vector.tensor_copy(wpoly[:, 2 * FS:3 * FS], mat[:])

            # ---------------- main loop over b blocks ----------------------
            rhs_v = rhs_all[:].rearrange("p (k c) -> p c k", k=NW)
            psum_all = ppool.tile([P, FS * NW], F32)

            # program-order-first / last matmuls on the psum_all bank
            active = [c for c in range(NBLK) if h1[c] > h0[c] or h1[c] < FS]
            c_first = active[0] if active else None
            c_last = active[-1] if active else None

            for rep in range(repeat):
                fr, lr = rep == 0, rep == repeat - 1
                for c in range(NBLK):
                    lo, mid = h0[c], h1[c]
                    if lo >= FS and mid >= FS:
                        continue
                    started = False
                    if mid > lo:
                        ind = ipool.tile([P, (mid - lo) * P], BF16,
                                         tag="ind", name="ind")
                        if c % 8 == 4:
                            # offload ~1/8 of indicator work to ACT:
                            # relu(sign(a-b)) is exactly (a > b) as 0/1
                            # (Sign and Relu share an activation table set)
                            sgn = ipool.tile([P, (mid - lo) * P], BF16,
                                             tag="sgn", name="sgn")
                            nc.scalar.activation(
                                sgn[:], a_bc[:, lo * P:mid * P], AF.Sign,
                                bias=bneg[:, c:c + 1],
                            )
                            nc.scalar.activation(ind[:], sgn[:], AF.Relu)
                        else:
                            nc.vector.tensor_scalar(
                                ind[:], a_bc[:, lo * P:mid * P],
                                bm[:, c:c + 1], None, op0=OP.is_gt,
                            )
                    for h in range(lo, mid):
                        st = fr and c == c_first and not started
                        started = True
                        sp = (lr and c == c_last and mid == FS and h == FS - 1)
                        nc.tensor.matmul(
                            psum_all[:, h * NW:(h + 1) * NW],
                            ind[:, (h - lo) * P:(h - lo + 1) * P],
                            rhs_v[:, c, :],
                            start=st,
                            stop=sp,
                        )
                    if mid < FS:
                        # all-1 suffix: one matmul with the ones weight and
                        # the block's [128, 7] weights repeated (stride-0)
                        nrep = FS - mid
                        rhs_rep = rhs_v[:, c:c + 1, :].broadcast_to(
                            (P, nrep, NW)
                        )
                        st = fr and c == c_first and not started
                        started = True
                        sp = lr and c == c_last
                        nc.tensor.matmul(
                            psum_all[:, mid * NW:FS * NW],
                            ones128[:],
                            rhs_rep,
                            start=st,
                            stop=sp,
                        )

            # ------------- bce over this core's N/8 rows (host sums) -------
            FC_ = N // NCORES // P
            zff = wpool.tile([P, 2 * FC_], F32)
            nc.gpsimd.dma_start(
                zff[:], yp_s[:].rearrange("(f p) c -> p f c", p=P)
            )
            zff_v = zff[:].rearrange("p (f c) -> p c f", c=2)
            ytb = wpool.tile([P, FC_], F32)
            nc.gpsimd.dma_start(ytb[:], yt_s[:].rearrange("(f p) -> p f", p=P))
            zf = wpool.tile([P, FC_], F32)
            nc.vector.tensor_sub(zf[:], zff_v[:, 1, :], zff_v[:, 0, :])
            pf = wpool.tile([P, FC_], F32)
            nc.scalar.activation(pf[:], zf[:], AF.Sigmoid)
            lp = wpool.tile([P, FC_], F32)
            nc.scalar.activation(lp[:], pf[:], AF.Ln)
            nc.vector.tensor_scalar(lp[:], lp[:], -100.0, None, op0=OP.max)
            q1 = wpool.tile([P, FC_], F32)
            nc.vector.tensor_scalar(q1[:], pf[:], -1.0, 1.0,
                                    op0=OP.mult, op1=OP.add)
            lq = wpool.tile([P, FC_], F32)
            nc.scalar.activation(lq[:], q1[:], AF.Ln)
            nc.vector.tensor_scalar(lq[:], lq[:], -100.0, None, op0=OP.max)
            dd = wpool.tile([P, FC_], F32)
            nc.vector.tensor_sub(dd[:], lp[:], lq[:])
            mmt = wpool.tile([P, FC_], F32)
            nc.vector.tensor_mul(mmt[:], dd[:], ytb[:])
            term = wpool.tile([P, FC_], F32)
            nc.vector.tensor_add(term[:], mmt[:], lq[:])

            # ---------------- epilogue -------------------------------------
            # per-chunk dot products on PE: out2[r, 3h+j] =
            #   sum_m psum[m, h*7+r] * wpoly[m, j*FS+h]
            # (j: 0 = a^2*mask, 1 = a*mask, 2 = mask), so
            #   S2 = sum_h out2[0,3h] + sum_{r=1..3} out2[r,3h+1]
            #               + sum_{r=4..6} out2[r,3h+2]
            #   C  = sum_h out2[0,3h+2]
            ps_sb = wpool.tile([P, FS * NW], F32)
            nc.vector.tensor_copy(ps_sb[:], psum_all[:])
            wp_v = wpoly[:].rearrange("p (j h) -> p h j", j=3)  # [128,FS,3]
            psum2 = ppool.tile([NW, 3 * FS], F32)
            for h in range(FS):
                nc.tensor.matmul(
                    psum2[:, 3 * h:3 * h + 3],
                    ps_sb[:, h * NW:(h + 1) * NW],
                    wp_v[:, h, :],
                    start=(h == 0),
                    stop=(h == FS - 1),
                )

            out_sb = wpool.tile([P, 8 + 3 * FS], F32)
            nc.vector.memset(out_sb[:], 0.0)
            nc.vector.tensor_reduce(
                out_sb[:, 2:3], term[:], axis=mybir.AxisListType.X, op=OP.add
            )
            nc.vector.tensor_copy(out_sb[0:NW, 8:8 + 3 * FS], psum2[:])
            nc.sync.dma_start(out[:], out_sb[:])

    nc.compile()
    return nc


_NC_CACHE = {}


def _get_nc(A, B, h0, h1):
    key = (A, B, tuple(h0), tuple(h1))
    if key not in _NC_CACHE:
        _NC_CACHE[key] = build_nc(A, B, h0, h1)
    return _NC_CACHE[key]


def _pad_up(n, m):
    return max(m, ((n + m - 1) // m) * m)


def make_plan(y_pred, y_true):
    """Host-side compaction + sort + staircase classification."""
    yp = np.ascontiguousarray(np.asarray(y_pred, dtype=np.float32))
    yt64 = np.asarray(y_true).astype(np.int64)
    yt = yt64.astype(np.float32)

    z = (yp[:, 1].astype(np.float64) - yp[:, 0].astype(np.float64))
    sig = 1.0 / (1.0 + np.exp(-z))

    neg_idx = np.where(yt64 == 1)[0]
    pos_idx = np.where(yt64 == 0)[0]
    neg_idx = neg_idx[np.argsort(z[neg_idx], kind="stable")]
    pos_idx = pos_idx[np.argsort(z[pos_idx], kind="stable")]
    nn, npos = len(neg_idx), len(pos_idx)

    B = _pad_up(npos, P)
    yp_b = np.zeros((B, 2), np.float32)
    yp_b[:npos] = yp[pos_idx]
    mb_v = np.ones((B,), np.float32)
    mb_v[:npos] = 0.0

    A = _pad_up((nn + NCORES - 1) // NCORES, P)
    FS, NBLK = A // P, B // P

    # f64 bounds per b block (sorted; pads -> B_PAD)
    bv = np.full((B,), B_PAD, np.float64)
    bv[:npos] = sig[pos_idx]
    b_lo = bv.reshape(NBLK, P).min(axis=1) - EPS
    b_hi = bv.reshape(NBLK, P).max(axis=1) + EPS

    # f64 bounds per a chunk, over the global rank window shared by all
    # cores (chunk h of core c holds sorted ranks {8*(128h+m)+c}); pads
    # (a = A_PAD, sorting last) appear on some core iff 8*128*(h+1) > nn.
    a_lo = np.empty(FS)
    a_hi = np.empty(FS)
    for h in range(FS):
        w0, w1 = 8 * P * h, min(8 * P * (h + 1), nn)
        if w1 > w0:
            win = sig[neg_idx[w0:w1]] + GAMMA
            lo, hi = win.min(), win.max()
        else:
            lo, hi = A_PAD, A_PAD
        if 8 * P * (h + 1) > nn:
            hi = max(hi, A_PAD)
        a_lo[h] = lo - EPS
        a_hi[h] = hi + EPS

    # staircase spans: per block, chunks [0,h0) all-0, [h0,h1) mixed,
    # [h1,FS) all-1
    h0 = np.empty(NBLK, np.int64)
    h1 = np.empty(NBLK, np.int64)
    for c in range(NBLK):
        h0[c] = np.searchsorted(a_hi, b_lo[c])        # a_hi[h] <= b_lo -> all-0
        g = np.nonzero(a_lo <= b_hi[c])[0]
        h1[c] = (g[-1] + 1) if len(g) else 0          # a_lo[h] > b_hi -> all-1
        if h1[c] < h0[c]:
            h1[c] = h0[c]

    maps = []
    for c in range(NCORES):
        sh = neg_idx[c::NCORES]
        yp_a = np.zeros((A, 2), np.float32)
        yp_a[: len(sh)] = yp[sh]
        ma_v = np.zeros((A,), np.float32)
        ma_v[: len(sh)] = 1.0
        mag_v = np.full((A,), A_PAD - 0.5, np.float32)
        mag_v[: len(sh)] = GAMMA
        sl = slice(c * (N // NCORES), (c + 1) * (N // NCORES))
        maps.append({
            "yp_a": yp_a, "ma": ma_v, "ma_g": mag_v,
            "yp_b": yp_b, "mb": mb_v,
            "yp_s": np.ascontiguousarray(yp[sl]),
            "yt_s": np.ascontiguousarray(yt[sl]),
            "idn": np.eye(P, dtype=np.float32),
        })
    return A, B, h0, h1, maps


def combine(outs):
    """outs: list of 8 [128, 8+3*FS] arrays -> scalar f32 loss."""
    s2 = 0.0
    cnt = 0.0
    bces = []
    for o in outs:
        o = o.astype(np.float64)
        bces.append(o[:, 2].sum())
        p2 = o[0:NW, 8:]
        fs = p2.shape[1] // 3
        for h in range(fs):
            s2 += p2[0, 3 * h] + p2[1:4, 3 * h + 1].sum() \
                  + p2[4:7, 3 * h + 2].sum()
            cnt += p2[0, 3 * h + 2]
    count = round(cnt)
    bce = -np.sum(bces) / N
    auc = s2 / max(count, 1)
    loss = ALPHA * bce + (1.0 - ALPHA) * auc if count > 0 else bce
    return np.array(loss, dtype=np.float32)


def run_hw(y_pred, y_true, trace=False, **kw):
    A, B, h0, h1, maps = make_plan(y_pred, y_true)
    nc = _get_nc(A, B, h0, h1)
    res = run_bass_kernel_spmd(nc, maps, list(range(NCORES)), trace=trace, **kw)
    outs = [res.results[c]["out"] for c in range(NCORES)]
    return combine(outs), res


def kernel(y_pred, y_true):
    loss, _ = run_hw(y_pred, y_true)
    return loss


if __name__ == "__main__":
    # local CoreSim self-test on each core's inputs
    from concourse.bass_interp import CoreSim

    rng = np.random.default_rng(0)
    y_pred = rng.standard_normal((N, 2), dtype=np.float32)
    y_true = rng.integers(0, 2, size=(N,)).astype(np.int64)

    A, B, h0, h1, maps = make_plan(y_pred, y_true)
    FS = A // P
    mixed = int((h1 - h0).sum())
    allone = int((FS - h1).sum())
    print(f"A={A} B={B} tiles: mixed={mixed} all1={allone} "
          f"all0={len(h0) * FS - mixed - allone}")
    nc = build_nc(A, B, h0, h1)

    pred = 1.0 / (1.0 + np.exp(-(y_pred[:, 1] - y_pred[:, 0]).astype(np.float64)))
    yt = y_true.astype(np.float64)
    lp = np.maximum(np.log(pred), -100)
    lq = np.maximum(np.log1p(-pred), -100)
    bce_all = yt * lp + (1 - yt) * lq
    neg_idx = np.where(y_true == 1)[0]
    pos_idx = np.where(y_true == 0)[0]
    b = pred[pos_idx]

    for core in range(2):
        sim = CoreSim(nc)
        for k, v in maps[core].items():
            sim.tensor(k)[:] = v
        sim.simulate(check_with_hw=False)
        o = np.array(sim.tensor("out")).astype(np.float64)

        zi = y_pred[:, 1].astype(np.float64) - y_pred[:, 0].astype(np.float64)
        order = neg_idx[np.argsort(zi[neg_idx], kind="stable")]
        a = pred[order[core::NCORES]] + GAMMA
        d = a[:, None] - b[None, :]
        msk = d > 0
        s2_ref = (np.where(msk, d, 0.0) ** 2).sum()
        k_ref = msk.sum()

        bce_ref = bce_all[core * (N // NCORES):(core + 1) * (N // NCORES)].sum()
        p2 = o[0:NW, 8:]
        s2_dev = sum(p2[0, 3 * h] + p2[1:4, 3 * h + 1].sum()
                     + p2[4:7, 3 * h + 2].sum() for h in range(FS))
        k_dev = sum(p2[0, 3 * h + 2] for h in range(FS))
        bce_dev = o[:, 2].sum()
        print(f"core{core}: S2 relerr={abs(s2_dev-s2_ref)/abs(s2_ref):.3e} "
              f"K err={k_dev-k_ref:.1f} "
              f"BCE relerr={abs(bce_dev-bce_ref)/abs(bce_ref):.3e}")



# revision 6
# speedup vs baseline: 8.2330x; 8.2330x over previous
"""AUC pairwise loss kernel for Trainium2, SPMD over 8 NeuronCores. v2.

Reference computation (N = 16384):
    pred = softmax(y_pred)[:, 1]                       # (N,)
    a_i  = pred_i + GAMMA   for rows with y_true == 1  ("neg" axis)
    b_j  = pred_j           for rows with y_true == 0  ("pos" axis)
    S2   = sum_{a_i > b_j} (a_i - b_j)^2,  C = #{a_i > b_j}
    auc  = S2 / max(C, 1)
    bce  = -mean(yt*clip(log pred, -100) + (1-yt)*clip(log(1-pred), -100))
    loss = ALPHA*bce + (1.0-ALPHA)*auc   (bce alone if C == 0)

Sharding: both classes are sorted by logit z1-z0 (monotone in pred).
"a" rows (yt==1) are dealt round-robin by sorted rank across 8 cores
(~1037 each, padded to A=1152); every core gets all "b" rows (yt==0,
padded to B=8192).  BCE is row-sharded N/8 per core.

With both sides sorted, ind[q, i] = (a_i > b_q) is a monotone
staircase.  For each 128-lane b block c the host finds the exact
uncertain a-range [lo_c, hi_c) from f64 sigmoid bounds over global
rank windows (margin EPS); measured widths are <= 31, so each block
gets a uniform W=32-col band at off_c = min(lo_c, A-W).  Columns left
of the band are certainly all-0 (skipped), columns right of
s_c = off_c + W are certainly all-1 (handled by a "step" matmul).

Device per-rep work:
  DVE: 2x tensor_tensor is_gt over the band layout [128, 32*W] f32
       (a broadcast along partitions, b -{} gamma expanded per block)
       -> bf16 indicator.
  PE:  3 step matmuls (stationary = hi/lo bf16 split of per-block
       weight column-sums [64, 14], moving = host 0/1 suffix matrix
       [64, bank]) that also zero psum rows 0:14 via start=True, then
       64 band matmuls (stationary = block weights [128, 7] bf16 =
       [w0 | -2b hi/mid/lo | b^2 hi/mid/lo], moving = indicator band
       [128, 32]) accumulating into psum[0:7, off_c:off_c+W].
       LDWEIGHTS cost ~ columns (7 or 14), so weight loads are ~6 ns.
Epilogue: DMA psum [14, A] and the BCE row-sums straight to DRAM; the
host applies the masked a-polynomials in f64:
  S2 = sum_i m_i (a_i^2 K_i + a_i S1_i + S2c_i),  C = sum_i m_i K_i
with K = rows 0+7, S1 = rows 1:4 + 8:11 (pre-scaled by -2),
S2c = rows 4:7 + 11:14.
"""

import numpy as np

from concourse import bacc, bass, mybir, tile
from concourse.bass_utils import run_bass_kernel_spmd

N = 16384
NCORES = 8
P = 128
GAMMA = 0.15
ALPHA = 0.6
A_PAD = 2.5                  # pad "a" slots sort last; masked out on host
EPS = 1e-4                   # host-vs-device sigmoid classification margin

F32 = mybir.dt.float32
BF16 = mybir.dt.bfloat16
AF = mybir.ActivationFunctionType
OP = mybir.AluOpType
NW = 7  # weight cols: w0, bh, bm, bl, b2h, b2m, b2l


def build_nc(A, B, W, off, skip, debug=False, repeat=1):
    """A: per-core padded 'a' count; B: padded 'b' count; W: uniform
    band width; off: per-b-block band start (len NBLK, monotone);
    skip: per-block bool, True -> block has no real b (no matmul).
    repeat>1 re-runs the main loop for slope benchmarking."""
    NBLK = B // P
    assert len(off) == NBLK and len(skip) == NBLK
    NB2 = (NBLK + 1) // 2
    halves = [(0, NB2), (NB2, NBLK)]
    BAND = NBLK * W

    nc = bacc.Bacc("TRN2", target_bir_lowering=False, debug=debug)

    z_band = nc.dram_tensor("z_band", [BAND], F32, kind="ExternalInput")
    yp_b = nc.dram_tensor("yp_b", [B, 2], F32, kind="ExternalInput")
    mb = nc.dram_tensor("mb", [B], F32, kind="ExternalInput")   # 1=pad
    s_mat = nc.dram_tensor("s_mat", [NBLK, A], BF16, kind="ExternalInput")
    yp_s = nc.dram_tensor("yp_s", [N // NCORES, 2], F32, kind="ExternalInput")
    yt_s = nc.dram_tensor("yt_s", [N // NCORES], F32, kind="ExternalInput")
    out_ps = nc.dram_tensor("out_ps", [2 * NW, A], F32, kind="ExternalOutput")
    out_bce = nc.dram_tensor("out_bce", [P, 1], F32, kind="ExternalOutput")
    # scratch for the colsum row->partition rearrange (SBUF->SBUF DMA
    # with partition scatter is broken on HW; DRAM roundtrip works)
    cs_scr = nc.dram_tensor("cs_scr", [NW * NBLK], F32, kind="ExternalOutput")

    # psum bank split points for the [*, A] accumulator (512 f32 / bank)
    banks = [(j, min(j + 512, A)) for j in range(0, A, 512)]

    with tile.TileContext(nc) as tc:
        with (
            tc.tile_pool(name="const", bufs=1) as cpool,
            tc.tile_pool(name="work", bufs=2) as wpool,
            tc.tile_pool(name="ind", bufs=2) as ipool,
            tc.tile_pool(name="psum", bufs=1, space=bass.MemorySpace.PSUM) as ppool,
            tc.tile_pool(name="psum_pro", bufs=2,
                         space=bass.MemorySpace.PSUM) as propool,
        ):
            # ---------------- band "a" row: sigmoid + broadcast -------------
            zrow = cpool.tile([1, BAND], F32)
            nc.sync.dma_start(zrow[:], z_band[:].rearrange("(o f) -> o f", o=1))
            srow = cpool.tile([1, BAND], F32)
            nc.scalar.activation(srow[:], zrow[:], AF.Sigmoid)

            ones1 = cpool.tile([1, P], F32)
            nc.vector.memset(ones1[:], 1.0)
            a_band = cpool.tile([P, BAND], F32)
            for j in range(0, BAND, 512):
                w = min(512, BAND - j)
                pb = propool.tile([P, 512], F32, name="pbc", tag="pbc")
                nc.tensor.matmul(pb[:, 0:w], ones1[:], srow[0:1, j:j + w],
                                 start=True, stop=True)
                nc.vector.tensor_copy(a_band[:, j:j + w], pb[:, 0:w])

            # ---------------- b side: pos pred, weights ---------------------
            zbb = wpool.tile([P, 2 * NBLK], F32)
            nc.gpsimd.dma_start(
                zbb[:], yp_b[:].rearrange("(f p) c -> p f c", p=P)
            )
            zbb_v = zbb[:].rearrange("p (f c) -> p c f", c=2)
            mbt = wpool.tile([P, NBLK], F32)
            nc.gpsimd.dma_start(mbt[:], mb[:].rearrange("(f p) -> p f", p=P))
            s_sb = cpool.tile([NBLK, A], BF16)
            nc.gpsimd.dma_start(s_sb[:], s_mat[:])

            zb = wpool.tile([P, NBLK], F32)
            nc.vector.tensor_sub(zb[:], zbb_v[:, 1, :], zbb_v[:, 0, :])
            bm = cpool.tile([P, NBLK], F32)
            nc.scalar.activation(bm[:], zb[:], AF.Sigmoid)
            # compare threshold b - gamma (gamma folded out of the a side)
            bmg = cpool.tile([P, NBLK], F32)
            nc.vector.tensor_scalar(bmg[:], bm[:], -GAMMA, None, op0=OP.add)
            # weight mask: 1 on real b, 0 on pads
            w0 = cpool.tile([P, NBLK], F32)
            nc.vector.tensor_scalar(w0[:], mbt[:], -1.0, 1.0,
                                    op0=OP.mult, op1=OP.add)

            # bf16 weights [w0 | -2bh | -2bm | -2bl | b2h | b2m | b2l];
            # the -2 scale (exact in bf16) pre-folds the cross term of
            # (a-b)^2.  b, b^2 masked to 0 on pad lanes before splitting.
            bz = cpool.tile([P, NBLK], F32)
            nc.vector.tensor_mul(bz[:], bm[:], w0[:])
            b2z = cpool.tile([P, NBLK], F32)
            nc.vector.tensor_mul(b2z[:], bz[:], bm[:])
            rhs_all = cpool.tile([P, NW * NBLK], BF16)
            nc.vector.tensor_copy(rhs_all[:, 0:NBLK], w0[:])
            for base, src, scl in ((1, bz, -2.0), (4, b2z, 1.0)):
                resid = src
                for k in range(3):
                    dst = rhs_all[:, (base + k) * NBLK:(base + k + 1) * NBLK]
                    nc.vector.tensor_scalar(dst, resid[:], scl, None,
                                            op0=OP.mult)
                    if k < 2:
                        back = wpool.tile([P, NBLK], F32, name=f"back{base}{k}",
                                          tag="back")
                        nc.scalar.mul(back[:], dst, 1.0 / scl)
                        nresid = wpool.tile([P, NBLK], F32, name=f"res{base}{k}",
                                            tag="resid")
                        nc.vector.tensor_sub(nresid[:], resid[:], back[:])
                        resid = nresid
            rhs_v = rhs_all[:].rearrange("p (k c) -> p c k", k=NW)

            # per-block weight column-sums -> step stationary [NBLK, 2*NW]
            # (hi/lo bf16 split of the f32 sums for ~18-bit precision)
            ones_c = cpool.tile([P, 1], BF16)
            nc.vector.memset(ones_c[:], 1.0)
            pcs = propool.tile([1, NW * NBLK], F32, name="pcs", tag="pcs")
            nc.tensor.matmul(pcs[:], ones_c[:], rhs_all[:],
                             start=True, stop=True)
            cs_row = wpool.tile([1, NW * NBLK], F32)
            nc.vector.tensor_copy(cs_row[:], pcs[:])
            cs64 = wpool.tile([NBLK, NW], F32)
            nc.sync.dma_start(cs_scr[:].rearrange("(o f) -> o f", o=1), cs_row[:])
            nc.sync.dma_start(
                cs64[:], cs_scr[:].rearrange("(r c) -> c r", c=NBLK)
            )
            cs2 = cpool.tile([NBLK, 2 * NW], BF16)
            nc.vector.tensor_copy(cs2[:, 0:NW], cs64[:])
            cs_back = wpool.tile([NBLK, NW], F32)
            nc.vector.tensor_copy(cs_back[:], cs2[:, 0:NW])
            cs_res = wpool.tile([NBLK, NW], F32)
            nc.vector.tensor_sub(cs_res[:], cs64[:], cs_back[:])
            nc.vector.tensor_copy(cs2[:, NW:2 * NW], cs_res[:])

            # expanded compare threshold bmx[p, c*W+j] = b[p, c] - gamma
            bmx = cpool.tile([P, BAND], F32)
            bmx_v = bmx[:].rearrange("p (c j) -> p c j", j=W)
            for c in range(NBLK):
                nc.vector.tensor_scalar(
                    bmx_v[:, c, :], bmg[:, c:c + 1].broadcast_to((P, W)),
                    1.0, None, op0=OP.mult,
                )

            # ---------------- main loop ------------------------------------
            a_v = a_band[:].rearrange("p (c j) -> p c j", j=W)
            psum = ppool.tile([2 * NW, A], F32)
            mm_blocks = [c for c in range(NBLK) if not skip[c]]
            last_c = mm_blocks[-1] if mm_blocks else None

            for rep in range(repeat):
                inds = []
                for half, (h0, h1) in enumerate(halves):
                    ind = ipool.tile([P, (h1 - h0) * W], BF16,
                                     name=f"ind{half}", tag="ind")
                    nc.vector.tensor_tensor(
                        ind[:].rearrange("p (c j) -> p c j", j=W),
                        a_v[:, h0:h1, :],
                        bmx_v[:, h0:h1, :],
                        op=OP.is_gt,
                    )
                    inds.append(ind)
                # step matmuls: all-1 suffix contribution, zero rows 0:14
                for b0, b1 in banks:
                    nc.tensor.matmul(
                        psum[:, b0:b1], cs2[:], s_sb[:, b0:b1],
                        start=True, stop=False, skip_group_check=True,
                    )
                # band matmuls
                for c in mm_blocks:
                    o = off[c]
                    half = 0 if c < NB2 else 1
                    ci = c - halves[half][0]
                    segs = []
                    for b0, b1 in banks:
                        s0, s1 = max(o, b0), min(o + W, b1)
                        if s0 < s1:
                            segs.append((s0, s1))
                    for s0, s1 in segs:
                        sp = (rep == repeat - 1 and c == last_c
                              and (s0, s1) == segs[-1])
                        nc.tensor.matmul(
                            psum[0:NW, s0:s1],
                            rhs_v[:, c, :],
                            inds[half][:, ci * W + (s0 - o):ci * W + (s1 - o)],
                            start=False, stop=sp, skip_group_check=True,
                        )

            # ------------- bce over this core's N/8 rows (host sums) -------
            FC_ = N // NCORES // P
            zff = wpool.tile([P, 2 * FC_], F32)
            nc.gpsimd.dma_start(
                zff[:], yp_s[:].rearrange("(f p) c -> p f c", p=P)
            )
            zff_v = zff[:].rearrange("p (f c) -> p c f", c=2)
            ytb = wpool.tile([P, FC_], F32)
            nc.gpsimd.dma_start(ytb[:], yt_s[:].rearrange("(f p) -> p f", p=P))
            zf = wpool.tile([P, FC_], F32)
            nc.vector.tensor_sub(zf[:], zff_v[:, 1, :], zff_v[:, 0, :])
            pf = wpool.tile([P, FC_], F32)
            nc.scalar.activation(pf[:], zf[:], AF.Sigmoid)
            lp = wpool.tile([P, FC_], F32)
            nc.scalar.activation(lp[:], pf[:], AF.Ln)
            nc.vector.tensor_scalar(lp[:], lp[:], -100.0, None, op0=OP.max)
            q1 = wpool.tile([P, FC_], F32)
            nc.vector.tensor_scalar(q1[:], pf[:], -1.0, 1.0,
                                    op0=OP.mult, op1=OP.add)
            lq = wpool.tile([P, FC_], F32)
            nc.scalar.activation(lq[:], q1[:], AF.Ln)
            nc.vector.tensor_scalar(lq[:], lq[:], -100.0, None, op0=OP.max)
            dd = wpool.tile([P, FC_], F32)
            nc.vector.tensor_sub(dd[:], lp[:], lq[:])
            mmt = wpool.tile([P, FC_], F32)
            nc.vector.tensor_mul(mmt[:], dd[:], ytb[:])
            term = wpool.tile([P, FC_], F32)
            nc.vector.tensor_add(term[:], mmt[:], lq[:])
            bce_sb = wpool.tile([P, 1], F32)
            nc.vector.tensor_reduce(
                bce_sb[:], term[:], axis=mybir.AxisListType.X, op=OP.add
            )

            # ---------------- outputs --------------------------------------
            ps_sb = wpool.tile([2 * NW, A], F32)
            nc.vector.tensor_copy(ps_sb[:, :A // 2], psum[:, :A // 2])
            nc.scalar.copy(ps_sb[:, A // 2:], psum[:, A // 2:])
            nc.sync.dma_start(out_ps[:], ps_sb[:])
            nc.sync.dma_start(out_bce[:], bce_sb[:])

    nc.compile()
    return nc


_NC_CACHE = {}


def _get_nc(A, B, W, off, skip):
    key = (A, B, W, tuple(off), tuple(skip))
    if key not in _NC_CACHE:
        _NC_CACHE[key] = build_nc(A, B, W, off, skip)
    return _NC_CACHE[key]


def _pad_up(n, m):
    return max(m, ((n + m - 1) // m) * m)


def make_plan(y_pred, y_true):
    """Host-side compaction + sort + band classification."""
    yp = np.ascontiguousarray(np.asarray(y_pred, dtype=np.float32))
    yt64 = np.asarray(y_true).astype(np.int64)
    yt = yt64.astype(np.float32)

    z = (yp[:, 1].astype(np.float64) - yp[:, 0].astype(np.float64))
    sig = 1.0 / (1.0 + np.exp(-z))

    neg_idx = np.where(yt64 == 1)[0]
    pos_idx = np.where(yt64 == 0)[0]
    neg_idx = neg_idx[np.argsort(z[neg_idx], kind="stable")]
    pos_idx = pos_idx[np.argsort(z[pos_idx], kind="stable")]
    nn, npos = len(neg_idx), len(pos_idx)

    B = _pad_up(npos, P)
    yp_b = np.zeros((B, 2), np.float32)
    yp_b[:npos] = yp[pos_idx]
    mb_v = np.ones((B,), np.float32)
    mb_v[:npos] = 0.0

    A = _pad_up((nn + NCORES - 1) // NCORES, P)
    NBLK = B // P

    # f64 a bounds per element, over the global rank window shared by
    # all cores (element i of core k holds sorted rank 8i+k); pads
    # (a = A_PAD, sorting last) appear on some core iff 8(i+1) > nn.
    av = np.full((NCORES * A,), A_PAD, np.float64)
    av[:nn] = sig[neg_idx] + GAMMA
    awin = av.reshape(A, NCORES)
    a_lo = awin.min(axis=1) - EPS
    a_hi = awin.max(axis=1) + EPS

    # f64 b bounds per block over REAL b only (pad weights are zeroed)
    bv = sig[pos_idx]
    b_lo = np.empty(NBLK)
    b_hi = np.empty(NBLK)
    skip = np.zeros(NBLK, bool)
    for c in range(NBLK):
        blk = bv[c * P:min((c + 1) * P, npos)]
        if len(blk) == 0:
            skip[c] = True
            b_lo[c] = b_hi[c] = 2.0
            continue
        b_lo[c] = blk.min() - EPS
        b_hi[c] = blk.max() + EPS

    # band [lo_c, hi_c): outside it the indicator is certainly 0 / 1
    lo = np.searchsorted(a_hi, b_lo, side="right")
    hi = np.searchsorted(a_lo, b_hi, side="left")
    wmax = int(np.max(np.maximum(hi - lo, 0), initial=0))
    W = max(16, _pad_up(wmax, 16))
    assert W <= min(A, 512)
    off = np.minimum(lo, A - W).astype(np.int64)
    off[skip] = 0
    s_end = off + W

    # suffix matrix: S[c, i] = 1 iff i >= s_end[c] (0 row if skip)
    import ml_dtypes
    s_np = (np.arange(A)[None, :] >= s_end[:, None]) & ~skip[:, None]
    s_np = s_np.astype(ml_dtypes.bfloat16)

    maps = []
    a_host = np.empty((NCORES, A), np.float64)
    m_host = np.zeros((NCORES, A), np.float64)
    for c in range(NCORES):
        sh = neg_idx[c::NCORES]
        a_host[c] = A_PAD
        a_host[c, :len(sh)] = sig[sh] + GAMMA
        m_host[c, :len(sh)] = 1.0
        zb_v = np.zeros((NBLK * W,), np.float32)
        for blk in range(NBLK):
            gi = off[blk] * NCORES + c + NCORES * np.arange(W)
            src = np.where(gi < nn, z[neg_idx[np.minimum(gi, nn - 1)]], 30.0)
            zb_v[blk * W:(blk + 1) * W] = src.astype(np.float32)
        sl = slice(c * (N // NCORES), (c + 1) * (N // NCORES))
        maps.append({
            "z_band": zb_v,
            "yp_b": yp_b, "mb": mb_v, "s_mat": np.ascontiguousarray(s_np),
            "yp_s": np.ascontiguousarray(yp[sl]),
            "yt_s": np.ascontiguousarray(yt[sl]),
        })
    return dict(A=A, B=B, W=W, off=off, skip=skip, maps=maps,
                a_host=a_host, m_host=m_host)


def combine(plan, res):
    """Apply masked a-polynomials to the psum partials (host, f64)."""
    s2 = 0.0
    cnt = 0.0
    bces = []
    for c in range(NCORES):
        o = res.results[c]
        ps = o["out_ps"].astype(np.float64)
        a = plan["a_host"][c]
        m = plan["m_host"][c]
        K = ps[0] + ps[NW]
        S1 = ps[1:4].sum(0) + ps[NW + 1:NW + 4].sum(0)
        S2c = ps[4:NW].sum(0) + ps[NW + 4:2 * NW].sum(0)
        s2 += float((m * (a * a * K + a * S1 + S2c)).sum())
        cnt += float((m * K).sum())
        bces.append(o["out_bce"].astype(np.float64).sum())
    count = round(cnt)
    bce = -np.sum(bces) / N
    auc = s2 / max(count, 1)
    loss = ALPHA * bce + (1.0 - ALPHA) * auc if count > 0 else bce
    return np.array(loss, dtype=np.float32)


def run_hw(y_pred, y_true, trace=False, **kw):
    plan = make_plan(y_pred, y_true)
    nc = _get_nc(plan["A"], plan["B"], plan["W"], plan["off"], plan["skip"])
    res = run_bass_kernel_spmd(nc, plan["maps"], list(range(NCORES)),
                               trace=trace, **kw)
    return combine(plan, res), res


def kernel(y_pred, y_true):
    loss, _ = run_hw(y_pred, y_true)
    return loss


if __name__ == "__main__":
    # local CoreSim self-test on each core's inputs
    from concourse.bass_interp import CoreSim

    rng = np.random.default_rng(0)
    y_pred = rng.standard_normal((N, 2), dtype=np.float32)
    y_true = rng.integers(0, 2, size=(N,)).astype(np.int64)

    plan = make_plan(y_pred, y_true)
    A, B, W = plan["A"], plan["B"], plan["W"]
    print(f"A={A} B={B} W={W} skip={plan['skip'].sum()}")
    nc = build_nc(A, B, W, plan["off"], plan["skip"])

    pred = 1.0 / (1.0 + np.exp(-(y_pred[:, 1] - y_pred[:, 0]).astype(np.float64)))
    yt = y_true.astype(np.float64)
    lp = np.maximum(np.log(pred), -100)
    lq = np.maximum(np.log1p(-pred), -100)
    bce_all = yt * lp + (1 - yt) * lq
    neg_idx = np.where(y_true == 1)[0]
    pos_idx = np.where(y_true == 0)[0]
    zi = y_pred[:, 1].astype(np.float64) - y_pred[:, 0].astype(np.float64)
    order = neg_idx[np.argsort(zi[neg_idx], kind="stable")]
    b = pred[pos_idx]

    class FakeRes:
        results = []

    for core in range(2):
        sim = CoreSim(nc)
        for k, v in plan["maps"][core].items():
            sim.tensor(k)[:] = v
        sim.simulate(check_with_hw=False)
        o = {"out_ps": np.array(sim.tensor("out_ps")),
             "out_bce": np.array(sim.tensor("out_bce"))}
        FakeRes.results.append(o)

        a = pred[order[core::NCORES]] + GAMMA
        d = a[:, None] - b[None, :]
        msk = d > 0
        s2_ref = (np.where(msk, d, 0.0) ** 2).sum()
        k_ref = msk.sum()
        bce_ref = bce_all[core * (N // NCORES):(core + 1) * (N // NCORES)].sum()

        ps = o["out_ps"].astype(np.float64)
        ah = plan["a_host"][core]
        mh = plan["m_host"][core]
        K = ps[0] + ps[NW]
        S1 = ps[1:4].sum(0) + ps[NW + 1:NW + 4].sum(0)
        S2c = ps[4:NW].sum(0) + ps[NW + 4:2 * NW].sum(0)
        s2_dev = float((mh * (ah * ah * K + ah * S1 + S2c)).sum())
        k_dev = float((mh * K).sum())
        bce_dev = o["out_bce"].astype(np.float64).sum()
        print(f"core{core}: S2 relerr={abs(s2_dev-s2_ref)/abs(s2_ref):.3e} "
              f"K err={k_dev-k_ref:.1f} "
              f"BCE relerr={abs(bce_dev-bce_ref)/abs(bce_ref):.3e}")
